# revision 1
# baseline (speedup 1.0000x reference)
"""Trainium2 Bass kernel for the NewTTS (Graves GMM-attention TTS decoder step) problem.

Strategy
--------
Data-parallel over the batch dim N=512 across 8 NeuronCores (64 rows each);
small weights replicated per core.

Key algorithmic observation: phi[n,u] = sum_k alpha*exp(-beta*(kappa-u)^2) is a
narrow Gaussian-mixture window.  For every u with beta*(u-kappa)^2 > ~104 each
exp() term underflows to exactly 0.0 in float32 (in the reference too), so
phi[:, u] == 0 and those u contribute nothing to wt = einsum(cx, phi).  We
compute a rigorous per-input cutoff U_cut on the host (float64 bound from the
same abk pre-activations the device recomputes exactly), slice cx to
[:, :U_cut, :], and have the device produce phi[:, :U_cut] / wt exactly.  The
remaining phi columns are exact zeros filled on the host.  This reduces the cx
traffic from 180 MB to ~3 MB.

Remaining DMA is dominated by the replicated GRU/projection weights; those
matmuls run in fp16 (weights and stationary operands cast on host / on ACT),
accumulating in fp32 PSUM.  The attention-parameter matmul (abk) stays in
full fp32 since kappa feeds exp().

Biases are folded into the matmuls as extra contraction rows against an
all-ones lhsT row, so gi/gh (+bias) come out of PSUM ready for the gate
nonlinearities.
"""

import numpy as np

import concourse.bacc as bacc
import concourse.bass as bass
import concourse.tile as tile
from concourse import mybir
from concourse.bass_utils import run_bass_kernel_spmd

F32 = mybir.dt.float32
F16 = mybir.dt.float16
AF = mybir.ActivationFunctionType
ALU = mybir.AluOpType
AX = mybir.AxisListType

NCORES = 8
N, UFULL, V = 512, 2048, 43
H, K = 512, 10
NS = N // NCORES  # 64 batch rows per core
CH = 32           # u-window chunk processed per iteration

EPS = 1e-5


def _bc(ap, counts):
    """Return `ap` with its free dims replaced by [step, count] pairs `counts`
    (step 0 = broadcast).  Partition dim is kept."""
    return bass.AP(tensor=ap.tensor, offset=ap.offset, ap=[ap.ap[0], *counts])


def _build(U):
    """Build + compile the per-core Bass program for a u-window of size U."""
    nch = U // CH
    nc = bacc.Bacc("TRN2", target_bir_lowering=False, debug=False,
                   num_devices=NCORES)

    # ---------------- DRAM I/O ----------------
    d_x1o = nc.dram_tensor("x1ones", [2, NS], F16, kind="ExternalInput")
    d_cx = nc.dram_tensor("cxs", [NS, U, V], F32, kind="ExternalInput")
    d_kap = nc.dram_tensor("kap", [NS, K], F32, kind="ExternalInput")
    d_h2p = nc.dram_tensor("h2p", [NS, H], F32, kind="ExternalInput")
    d_h3p = nc.dram_tensor("h3p", [NS, H], F32, kind="ExternalInput")
    d_h2pT = nc.dram_tensor("h2pT", [H, NS], F16, kind="ExternalInput")
    d_h3pT = nc.dram_tensor("h3pT", [H, NS], F16, kind="ExternalInput")
    d_XTa = nc.dram_tensor("XTa", [H + V + 1, NS], F32, kind="ExternalInput")
    d_W1Ta = nc.dram_tensor("W1Ta", [H + V + 1, 3 * K], F32, kind="ExternalInput")
    d_wihA = nc.dram_tensor("wihA", [V, 3 * H], F16, kind="ExternalInput")
    d_wihB = nc.dram_tensor("wihB", [2, 3 * H], F16, kind="ExternalInput")
    d_whh2 = nc.dram_tensor("whh2", [H, 3 * H], F16, kind="ExternalInput")
    d_whh2b = nc.dram_tensor("whh2b", [1, 3 * H], F16, kind="ExternalInput")
    d_wih3 = nc.dram_tensor("wih3", [H, 3 * H], F16, kind="ExternalInput")
    d_wih3b = nc.dram_tensor("wih3b", [1, 3 * H], F16, kind="ExternalInput")
    d_whh3 = nc.dram_tensor("whh3", [H, 3 * H], F16, kind="ExternalInput")
    d_whh3b = nc.dram_tensor("whh3b", [1, 3 * H], F16, kind="ExternalInput")
    d_Y = nc.dram_tensor("Yw", [H, H], F16, kind="ExternalInput")
    d_Yb = nc.dram_tensor("Yb", [1, H], F16, kind="ExternalInput")

    d_dist = nc.dram_tensor("dist_o", [NS, H], F32, kind="ExternalOutput")
    d_h2o = nc.dram_tensor("h2_o", [NS, H], F32, kind="ExternalOutput")
    d_h3o = nc.dram_tensor("h3_o", [NS, H], F32, kind="ExternalOutput")
    d_phio = nc.dram_tensor("phi_o", [NS, U], F32, kind="ExternalOutput")
    d_wto = nc.dram_tensor("wt_o", [NS, V], F32, kind="ExternalOutput")
    d_kapo = nc.dram_tensor("kappa_o", [NS, K], F32, kind="ExternalOutput")

    # u-coordinate grid, replicated per (partition, k); baked into the NEFF
    d_iota = nc.inline_tensor(
        np.tile(np.arange(U, dtype=np.float32), (NS, K, 1)), name="iotac")
    d_eye = nc.inline_tensor(np.eye(NS, dtype=np.float16), name="eye16")

    with tile.TileContext(nc) as tc, \
            tc.tile_pool(name="wp", bufs=1) as wp, \
            tc.tile_pool(name="wk", bufs=2) as wk, \
            tc.tile_pool(name="psA", bufs=1, space="PSUM") as psA, \
            tc.tile_pool(name="psT", bufs=2, space="PSUM") as psT, \
            tc.tile_pool(name="psG", bufs=1, space="PSUM") as psG:

        # ---------------- weight / input loads ----------------
        def load(dram, shape, dtype, rearr=None):
            t = wp.tile(shape, dtype, name=dram.name + "_sb")
            src = dram.ap()
            if rearr is not None:
                src = src.rearrange(rearr, p=128)
            nc.sync.dma_start(out=t, in_=src)
            return t

        wihA_sb = load(d_wihA, [V, 3 * H], F16)
        wihB_sb = load(d_wihB, [2, 3 * H], F16)
        whh2_sb = load(d_whh2, [128, 4, 3 * H], F16, "(c p) e -> p c e")
        whh2b_sb = load(d_whh2b, [1, 3 * H], F16)
        wih3_sb = load(d_wih3, [128, 4, 3 * H], F16, "(c p) e -> p c e")
        wih3b_sb = load(d_wih3b, [1, 3 * H], F16)
        whh3_sb = load(d_whh3, [128, 4, 3 * H], F16, "(c p) e -> p c e")
        whh3b_sb = load(d_whh3b, [1, 3 * H], F16)
        Y_sb = load(d_Y, [128, 4, H], F16, "(c p) e -> p c e")
        Yb_sb = load(d_Yb, [1, H], F16)
        h2pT_sb = load(d_h2pT, [128, 4, NS], F16, "(c p) e -> p c e")
        h3pT_sb = load(d_h3pT, [128, 4, NS], F16, "(c p) e -> p c e")
        h2p_sb = load(d_h2p, [NS, H], F32)
        h3p_sb = load(d_h3p, [NS, H], F32)
        kap_sb = load(d_kap, [NS, K], F32)
        x1o_sb = load(d_x1o, [2, NS], F16)
        eye_sb = load(d_eye, [NS, NS], F16)
        iota_sb = load(d_iota, [NS, K, U], F32)

        xta_sb = wp.tile([128, 5, NS], F32)
        w1t_sb = wp.tile([128, 5, 3 * K], F32)
        for c in range(5):
            cnt = 128 if c < 4 else (H + V + 1 - 512)
            nc.sync.dma_start(out=xta_sb[:cnt, c, :],
                              in_=d_XTa.ap()[c * 128:c * 128 + cnt, :])
            nc.sync.dma_start(out=w1t_sb[:cnt, c, :],
                              in_=d_W1Ta.ap()[c * 128:c * 128 + cnt, :])

        ones16 = wp.tile([1, NS], F16)
        nc.vector.memset(ones16, 1.0)

        # ---------------- stage A: abk = [eh, wt_1, 1] @ [W.T; b] ----------------
        ps_abk = psA.tile([NS, 3 * K], F32)
        for c in range(5):
            cnt = 128 if c < 4 else (H + V + 1 - 512)
            nc.tensor.matmul(ps_abk, xta_sb[:cnt, c, :], w1t_sb[:cnt, c, :],
                             start=(c == 0), stop=(c == 4))

        a_sb = wk.tile([NS, K], F32)
        nc.scalar.copy(a_sb, ps_abk[:, 0:K])
        beta_sb = wk.tile([NS, K], F32)
        nc.scalar.activation(beta_sb, ps_abk[:, K:2 * K], AF.Exp)
        nbeta_sb = wk.tile([NS, K], F32)
        # -(exp(b) + EPS)
        nc.vector.tensor_scalar(nbeta_sb, beta_sb, -1.0, -EPS, ALU.mult, ALU.add)
        kexp_sb = wk.tile([NS, K], F32)
        nc.scalar.activation(kexp_sb, ps_abk[:, 2 * K:3 * K], AF.Exp)
        kappa_sb = wk.tile([NS, K], F32)
        nc.vector.tensor_add(kappa_sb, kexp_sb, kap_sb)
        nc.sync.dma_start(out=d_kapo.ap(), in_=kappa_sb)
        negkap_sb = wk.tile([NS, K], F32)
        nc.vector.tensor_scalar_mul(negkap_sb, kappa_sb, -1.0)

        # ---------------- phi + wt, per u-chunk ----------------
        # phi[n,u] = sum_k exp(a_k - (exp(b_k)+eps) * (u - kappa_k)^2)
        # (alpha's +eps dropped: relative error ~1e-5)
        phi_sb = wk.tile([NS, U], F32, bufs=1)
        wt_sb = wk.tile([NS, V], F32, bufs=1)
        for c in range(nch):
            u0 = c * CH
            io = iota_sb[:, :, u0:u0 + CH]                       # [NS, K, CH]
            dd = wk.tile([NS, K, CH], F32, tag="dd")
            nc.vector.tensor_add(dd, io, _bc(negkap_sb, [[1, K], [0, CH]]))
            d2 = wk.tile([NS, K, CH], F32, tag="d2")
            nc.vector.tensor_mul(d2, dd, dd)
            ar = wk.tile([NS, K, CH], F32, tag="ar")
            nc.vector.tensor_mul(ar, d2, _bc(nbeta_sb, [[1, K], [0, CH]]))
            ar2 = wk.tile([NS, K, CH], F32, tag="ar2")
            nc.vector.tensor_add(ar2, ar, _bc(a_sb, [[1, K], [0, CH]]))
            ee = wk.tile([NS, K, CH], F32, tag="ee")
            nc.scalar.activation(ee, ar2, AF.Exp)
            # sum over k (view: innermost = k)
            eev = bass.AP(tensor=ee.tensor, offset=ee.offset,
                          ap=[ee.ap[0], [1, CH], [CH, K]])
            nc.vector.tensor_reduce(phi_sb[:, u0:u0 + CH], eev,
                                    axis=AX.X, op=ALU.add)

            # wt += einsum('nuv,nu->nv') over this chunk
            cxt = wk.tile([NS, CH, V], F32, tag="cxt")
            nc.sync.dma_start(out=cxt, in_=d_cx.ap()[:, u0:u0 + CH, :])
            prod = wk.tile([NS, V, CH], F32, tag="prod")
            cxv = bass.AP(tensor=cxt.tensor, offset=cxt.offset,
                          ap=[cxt.ap[0], [1, V], [V, CH]])
            ph = phi_sb[:, u0:u0 + CH]
            phb = bass.AP(tensor=ph.tensor, offset=ph.offset,
                          ap=[ph.ap[0], [0, V], [1, CH]])
            nc.vector.tensor_mul(prod, cxv, phb)
            if c == 0:
                nc.vector.tensor_reduce(wt_sb, prod, axis=AX.X, op=ALU.add)
            else:
                wtp = wk.tile([NS, V], F32, tag="wtp")
                nc.vector.tensor_reduce(wtp, prod, axis=AX.X, op=ALU.add)
                nc.vector.tensor_add(wt_sb, wt_sb, wtp)
        nc.sync.dma_start(out=d_phio.ap(), in_=phi_sb)
        nc.sync.dma_start(out=d_wto.ap(), in_=wt_sb)

        # ---------------- GRU cells ----------------
        def make_gates(gi_terms, gh_terms):
            ps_r = psG.tile([NS, H], F32, tag="ps_r")
            ps_z = psG.tile([NS, H], F32, tag="ps_z")
            ps_gin = psG.tile([NS, H], F32, tag="ps_gin")
            ps_ghn = psG.tile([NS, H], F32, tag="ps_ghn")
            for g, ps in ((0, ps_r), (1, ps_z)):
                terms = gi_terms + gh_terms
                for i, (l, r) in enumerate(terms):
                    nc.tensor.matmul(ps, l, r[:, g * H:(g + 1) * H],
                                     start=(i == 0), stop=(i == len(terms) - 1))
            for i, (l, r) in enumerate(gi_terms):
                nc.tensor.matmul(ps_gin, l, r[:, 2 * H:3 * H],
                                 start=(i == 0), stop=(i == len(gi_terms) - 1))
            for i, (l, r) in enumerate(gh_terms):
                nc.tensor.matmul(ps_ghn, l, r[:, 2 * H:3 * H],
                                 start=(i == 0), stop=(i == len(gh_terms) - 1))
            return ps_r, ps_z, ps_gin, ps_ghn

        def gru_tail(ps_r, ps_z, ps_gin, ps_ghn, hprev_sb):
            r_sb = wk.tile([NS, H], F32, tag="r_sb")
            nc.scalar.activation(r_sb, ps_r, AF.Sigmoid)
            z_sb = wk.tile([NS, H], F32, tag="z_sb")
            nc.scalar.activation(z_sb, ps_z, AF.Sigmoid)
            rg = wk.tile([NS, H], F32, tag="rg")
            nc.vector.tensor_mul(rg, r_sb, ps_ghn)
            npre = wk.tile([NS, H], F32, tag="npre")
            nc.vector.tensor_add(npre, rg, ps_gin)
            n_sb = wk.tile([NS, H], F32, tag="n_sb")
            nc.scalar.activation(n_sb, npre, AF.Tanh)
            # h = n + z*(hprev - n)
            hmn = wk.tile([NS, H], F32, tag="hmn")
            nc.vector.tensor_sub(hmn, hprev_sb, n_sb)
            zd = wk.tile([NS, H], F32, tag="zd")
            nc.vector.tensor_mul(zd, z_sb, hmn)
            h_sb = wk.tile([NS, H], F32, tag="h_sb")
            nc.vector.tensor_add(h_sb, n_sb, zd)
            return h_sb

        def transpose4(src16, dst_name):
            dst = wk.tile([128, 4, NS], F16, tag=dst_name, bufs=1)
            for c in range(4):
                ptp = psT.tile([128, NS], F16, tag="ptp", bufs=2)
                nc.tensor.transpose(ptp, src16[:, c * 128:(c + 1) * 128], eye_sb)
                nc.vector.tensor_copy(dst[:, c, :], ptp)
            return dst

        # GRU2: x2 = [xt, wt]
        wt16 = wk.tile([NS, V], F16)
        nc.vector.tensor_copy(wt16, wt_sb)
        pwtT = psT.tile([V, NS], F16, tag="ptp", bufs=2)
        nc.tensor.transpose(pwtT, wt16, eye_sb)
        wtT_sb = wk.tile([V, NS], F16)
        nc.vector.tensor_copy(wtT_sb, pwtT)

        gi2 = [(wtT_sb, wihA_sb), (x1o_sb, wihB_sb)]
        gh2 = [(h2pT_sb[:, c, :], whh2_sb[:, c, :]) for c in range(4)] \
            + [(ones16, whh2b_sb)]
        h2_sb = gru_tail(*make_gates(gi2, gh2), h2p_sb)
        nc.sync.dma_start(out=d_h2o.ap(), in_=h2_sb)

        rh16 = wk.tile([NS, H], F16)
        nc.scalar.activation(rh16, h2_sb, AF.Relu)
        rh2T = transpose4(rh16, "rh2T")

        gi3 = [(rh2T[:, c, :], wih3_sb[:, c, :]) for c in range(4)] \
            + [(ones16, wih3b_sb)]
        gh3 = [(h3pT_sb[:, c, :], whh3_sb[:, c, :]) for c in range(4)] \
            + [(ones16, whh3b_sb)]
        h3_sb = gru_tail(*make_gates(gi3, gh3), h3p_sb)
        nc.sync.dma_start(out=d_h3o.ap(), in_=h3_sb)

        # out = h3 + h2 ; dist = relu(out) @ Y.T + Yb
        o_sb = wk.tile([NS, H], F32)
        nc.vector.tensor_add(o_sb, h3_sb, h2_sb)
        ro16 = wk.tile([NS, H], F16)
        nc.scalar.activation(ro16, o_sb, AF.Relu)
        roT = transpose4(ro16, "roT")

        ps_d = psG.tile([NS, H], F32, tag="ps_d")
        yterms = [(roT[:, c, :], Y_sb[:, c, :]) for c in range(4)] \
            + [(ones16, Yb_sb)]
        for i, (l, r) in enumerate(yterms):
            nc.tensor.matmul(ps_d, l, r, start=(i == 0),
                             stop=(i == len(yterms) - 1))
        dist_sb = wk.tile([NS, H], F32)
        nc.scalar.copy(dist_sb, ps_d)
        nc.sync.dma_start(out=d_dist.ap(), in_=dist_sb)

    nc.compile()
    return nc


# ---------------------------------------------------------------------------


def _u_cut(inputs):
    """Smallest u-window (multiple of CH) outside which every phi term
    underflows to exact f32 zero (rigorous float64 bound, computed from the
    same quantities the device recomputes exactly)."""
    eh = np.asarray(inputs["encoder_hidden"], np.float64)[0]
    wt1 = np.asarray(inputs["wt_1"], np.float64)
    W1 = np.asarray(inputs["Wh1p_w"], np.float64)
    b1 = np.asarray(inputs["Wh1p_b"], np.float64)
    kap = np.asarray(inputs["kappa_t_1"], np.float64)
    abk = np.concatenate([eh, wt1], 1) @ W1.T + b1
    a = abk[:, :K]
    beta = np.exp(abk[:, K:2 * K])          # +EPS only shrinks the window
    kappa = kap + np.exp(abk[:, 2 * K:3 * K])
    # need beta*(u-kappa)^2 - a > ~104 for exp() -> exact 0 in f32
    need = kappa + np.sqrt((106.0 + np.maximum(a, 0.0)) / beta)
    U = int(np.ceil(need.max())) + 2
    U = max(CH, ((U + CH - 1) // CH) * CH)
    return min(U, UFULL)


def _prep(inputs, U):
    f32 = lambda x: np.asarray(x, dtype=np.float32)
    xt = f32(inputs["xt"])
    cx = f32(inputs["cx"])
    eh = f32(inputs["encoder_hidden"])[0]
    wt1 = f32(inputs["wt_1"])
    h2p = f32(inputs["h2t_1"])[0]
    h3p = f32(inputs["h3t_1"])[0]
    kap = f32(inputs["kappa_t_1"])

    W1Ta = np.ascontiguousarray(
        np.concatenate([f32(inputs["Wh1p_w"]).T,
                        f32(inputs["Wh1p_b"])[None, :]], 0))
    wih2T = f32(inputs["rnn2_wih"]).T          # [44, 1536]
    wihA = np.ascontiguousarray(wih2T[1:]).astype(np.float16)
    wihB = np.ascontiguousarray(
        np.stack([wih2T[0], f32(inputs["rnn2_bih"])])).astype(np.float16)
    whh2 = np.ascontiguousarray(f32(inputs["rnn2_whh"]).T).astype(np.float16)
    whh2b = f32(inputs["rnn2_bhh"])[None, :].astype(np.float16)
    wih3 = np.ascontiguousarray(f32(inputs["rnn3_wih"]).T).astype(np.float16)
    wih3b = f32(inputs["rnn3_bih"])[None, :].astype(np.float16)
    whh3 = np.ascontiguousarray(f32(inputs["rnn3_whh"]).T).astype(np.float16)
    whh3b = f32(inputs["rnn3_bhh"])[None, :].astype(np.float16)
    Yw = np.ascontiguousarray(f32(inputs["Y_w"]).T).astype(np.float16)
    Yb = f32(inputs["Y_b"])[None, :].astype(np.float16)

    shared = dict(W1Ta=W1Ta, wihA=wihA, wihB=wihB, whh2=whh2, whh2b=whh2b,
                  wih3=wih3, wih3b=wih3b, whh3=whh3, whh3b=whh3b,
                  Yw=Yw, Yb=Yb)

    ones = np.ones((1, NS), np.float32)
    in_maps = []
    for s in range(NCORES):
        sl = slice(s * NS, (s + 1) * NS)
        in_maps.append(dict(
            x1ones=np.concatenate([xt[sl][None, :], ones], 0
                                  ).astype(np.float16),
            cxs=np.ascontiguousarray(cx[sl, :U, :]),
            kap=np.ascontiguousarray(kap[sl]),
            h2p=np.ascontiguousarray(h2p[sl]),
            h3p=np.ascontiguousarray(h3p[sl]),
            h2pT=np.ascontiguousarray(h2p[sl].T).astype(np.float16),
            h3pT=np.ascontiguousarray(h3p[sl].T).astype(np.float16),
            XTa=np.ascontiguousarray(
                np.concatenate([eh[sl].T, wt1[sl].T, ones], 0)),
            **shared,
        ))
    return in_maps


_cache = {}


def _get_nc(U):
    if U not in _cache:
        _cache[U] = _build(U)
    return _cache[U]


def _gather(results, U):
    cat = lambda name: np.concatenate([r[name] for r in results], 0)
    dist = cat("dist_o")[:, None, :]
    h2 = cat("h2_o")[None]
    h3 = cat("h3_o")[None]
    phi = np.zeros((N, UFULL), np.float32)
    phi[:, :U] = cat("phi_o")
    wt = cat("wt_o")
    kappa = cat("kappa_o")
    return dist, h2, h3, phi, wt, kappa


def kernel(**inputs):
    U = _u_cut(inputs)
    nc = _get_nc(U)
    in_maps = _prep(inputs, U)
    res = run_bass_kernel_spmd(nc, in_maps, core_ids=list(range(NCORES)))
    return _gather(res.results, U)


def run_traced(inputs):
    """test.py helper: run with NTFF tracing, return (outputs, BassKernelResults)."""
    U = _u_cut(inputs)
    nc = _get_nc(U)
    in_maps = _prep(inputs, U)
    res = run_bass_kernel_spmd(nc, in_maps, core_ids=list(range(NCORES)),
                               trace=True, trace_cores=list(range(NCORES)))
    return _gather(res.results, U), res
